# revision 12
# baseline (speedup 1.0000x reference)
"""DANet-style channel attention kernel for Trainium2 (8 NeuronCores).

Problem (hardcoded): B=16, C=256, H=W=128 (N=HW=16384), fp32.
  q = Wq@Q+bq; k = Wk@K+bk; v = Wv@X+bv          (1x1 convs, per batch elem)
  energy = q @ k^T            [C,C]
  attn   = softmax(rowmax(energy) - energy)       (== softmax(-energy))
  out    = attn @ v           [C,N]

Key algebraic reformulation (v2): the v conv is never materialized.
  out = attn @ (Wv X + bv 1^T) = (attn Wv) @ X + (attn bv) 1^T
so instead of computing v = Wv X over all N pixels (C^2 N MACs) and then
attn @ v (another C^2 N), we compute G = attn Wv (a C^3 = 16.7M MAC nit)
and a single streamed GEMM out = G @ X + pbv (C^2 N).  This removes 20% of
the PE work, the fp16 v residency (64 KiB/partition of SBUF), and the ACT
copies, and moves the x stream from phase A to phase D.

Sharding: data-parallel over batch; 2 batch elements per core, 8 cores.

Per-core structure (per batch element; phases of adjacent elements overlap):
  A: stream q,k in 1 MiB chunks (q->SP ring, k->ACT ring); per 512-px chunk
     produce qT/kT tiles [n128 x f256] directly in transposed layout (the
     input tile is the PE stationary operand, W^T the moving operand -> no
     transposes anywhere), add biases via one DVE tensor_add per chunk, and
     accumulate the full energy [256,256] in one persistent PSUM bank.
  B: rowmin via DVE reduce(min); P = Exp(-energy + rowmin) on ACT with fused
     row-sum (accum_out); pbv = P @ bv via one fused DVE
     tensor_tensor_reduce per half; PE-transpose of the four 128x128 blocks
     -> PT; G^T = Wv^T @ PT (4 small matmuls); rinv = 1/rowsum.
  D: stream x in 2 MiB chunks; out = (G^T.T @ x + pbv) * rinv with the
     bias-add and normalization fused into one DVE tensor_scalar per 512-px
     half; streamed to DRAM in 2 MiB stores on SWDGE.

Matmuls run as float32r (FP22: fp32 bytes, truncated mantissa; full PE rate
at free-dim >= 256).  End-to-end relative error vs the fp32 reference:
~2e-3.

PSUM budget (8 banks): qt 2 + kt 2 + energy/PT 1 + G 1 + out 2.

Walrus constraint handled here: a fused-LDW (4-byte dtype) matmul carries at
most ONE semaphore wait, and bass'es legalization for that lives in Bacc
(generate_event_semaphores), so the module is built with bacc.Bacc() and
finalized before execution.

Timing (hw_time.py, For_i-loop slope method on 8 axon trn2 cores).
"""

import numpy as np

B_FULL = 16
N_CORES = 8
B2 = B_FULL // N_CORES  # batch elems per core
C = 256
N = 16384  # H*W
CH_DA = 1024  # phase-A DMA chunk (pixels) -> 1 MiB per load
CH_DX = 2048  # phase-D x-load chunk (pixels) -> 2 MiB per load
CH_DO = 2048  # phase-D store chunk (pixels) -> 2 MiB per store
CH_A = 512    # phase-A compute chunk (pixels)

_CACHE = {}


def _build(loop=None):
    import contextlib

    import concourse.bass as bass
    import concourse.tile as tile
    from concourse import bacc, mybir

    f32 = mybir.dt.float32
    f32r = mybir.dt.float32r
    AF = mybir.ActivationFunctionType
    AX = mybir.AxisListType
    OP = mybir.AluOpType

    nc = bacc.Bacc()

    q_in = nc.declare_dram_parameter("q_in", [B2, C, N], f32r, isOutput=False)
    k_in = nc.declare_dram_parameter("k_in", [B2, C, N], f32r, isOutput=False)
    x_in = nc.declare_dram_parameter("x_in", [B2, C, N], f32r, isOutput=False)
    wqt_d = nc.declare_dram_parameter("wqt", [C, C], f32r, isOutput=False)
    wkt_d = nc.declare_dram_parameter("wkt", [C, C], f32r, isOutput=False)
    wv_d = nc.declare_dram_parameter("wv", [C, C], f32r, isOutput=False)
    bqb_d = nc.declare_dram_parameter("bqb", [128, 4, 256], f32, isOutput=False)
    bkb_d = nc.declare_dram_parameter("bkb", [128, 4, 256], f32, isOutput=False)
    bvb_d = nc.declare_dram_parameter("bvb", [128, 256], f32, isOutput=False)
    id_d = nc.declare_dram_parameter("ident", [128, 128], f32, isOutput=False)
    out_d = nc.declare_dram_parameter("out", [B2, C, N], f32, isOutput=True)

    with tile.TileContext(nc) as tc:
        with (
            tc.tile_pool(name="const", bufs=1) as const,
            tc.tile_pool(name="qkc", bufs=3) as qkc,
            tc.tile_pool(name="xc_p", bufs=2) as xc_p,
            tc.tile_pool(name="tsb", bufs=3) as tsb,
            tc.tile_pool(name="osb", bufs=2) as osb,
            tc.tile_pool(name="smax", bufs=2) as smax,
            tc.tile_pool(name="ps_qt", bufs=1, space="PSUM") as ps_qt,
            tc.tile_pool(name="ps_kt", bufs=1, space="PSUM") as ps_kt,
            tc.tile_pool(name="ps_e", bufs=1, space="PSUM") as ps_e,
            tc.tile_pool(name="ps_g", bufs=1, space="PSUM") as ps_g,
            tc.tile_pool(name="ps_o", bufs=2, space="PSUM") as ps_o,
        ):
            # ---- constants ----
            wqt = const.tile([128, 2, C], f32r)
            wkt = const.tile([128, 2, C], f32r)
            wv = const.tile([128, 2, C], f32r)
            for w_sb, w_d in ((wqt, wqt_d), (wkt, wkt_d), (wv, wv_d)):
                nc.sync.dma_start(
                    out=w_sb[:, :, :],
                    in_=w_d[:, :].rearrange("(t p) f -> p t f", p=128))
            bqb = const.tile([128, 4, 256], f32)
            bkb = const.tile([128, 4, 256], f32)
            bvb = const.tile([128, 256], f32)
            ident = const.tile([128, 128], f32)
            nc.sync.dma_start(out=bqb[:, :, :], in_=bqb_d[:, :, :])
            nc.sync.dma_start(out=bkb[:, :, :], in_=bkb_d[:, :, :])
            nc.sync.dma_start(out=bvb[:, :], in_=bvb_d[:, :])
            nc.sync.dma_start(out=ident[:, :], in_=id_d[:, :])

            n_sub_a = CH_A // 128
            loop_cm = tc.For_i(0, loop) if loop else contextlib.nullcontext()
            with loop_cm:
              for b in range(B2):
                # == phase A: stream q,k; accumulate energy in PSUM =======
                e_ps = ps_e.tile([128, 2, 256], f32, tag="e")
                for cd in range(N // CH_DA):
                  qc = qkc.tile([128, 2, CH_DA], f32r, tag="qc")
                  kc = qkc.tile([128, 2, CH_DA], f32r, tag="kc")
                  base = cd * CH_DA
                  nc.sync.dma_start(
                      out=qc[:, :, :],
                      in_=q_in[b, :, base:base + CH_DA].rearrange(
                          "(t p) n -> p t n", p=128))
                  nc.scalar.dma_start(
                      out=kc[:, :, :],
                      in_=k_in[b, :, base:base + CH_DA].rearrange(
                          "(t p) n -> p t n", p=128))
                  for cc in range(CH_DA // CH_A):
                    ci = cd * (CH_DA // CH_A) + cc
                    co = cc * CH_A  # offset within the DMA chunk
                    qt_sb = tsb.tile([128, n_sub_a, 256], f32r, tag="qt_sb")
                    kt_sb = tsb.tile([128, n_sub_a, 256], f32r, tag="kt_sb")
                    qt_ps = ps_qt.tile([128, n_sub_a, 256], f32)
                    kt_ps = ps_kt.tile([128, n_sub_a, 256], f32)
                    for ns in range(n_sub_a):
                        for ct in range(2):
                            nc.tensor.matmul(
                                qt_ps[:, ns, :],
                                lhsT=qc[:, ct, co + ns * 128:
                                        co + (ns + 1) * 128],
                                rhs=wqt[:, ct, :],
                                start=(ct == 0 and ns % 2 == 0),
                                stop=(ct == 1),
                                skip_group_check=True)
                        for ct in range(2):
                            nc.tensor.matmul(
                                kt_ps[:, ns, :],
                                lhsT=kc[:, ct, co + ns * 128:
                                        co + (ns + 1) * 128],
                                rhs=wkt[:, ct, :],
                                start=(ct == 0 and ns % 2 == 0),
                                stop=(ct == 1),
                                skip_group_check=True)
                    # single bias add (broadcast along partitions) + to SBUF
                    nc.vector.tensor_add(
                        qt_sb[:, :, :], qt_ps[:, :, :], bqb[:, :, :])
                    nc.vector.tensor_add(
                        kt_sb[:, :, :], kt_ps[:, :, :], bkb[:, :, :])
                    # energy += qT^T @ kT
                    for ns in range(n_sub_a):
                        for cm in range(2):
                            nc.tensor.matmul(
                                e_ps[:, cm, :],
                                lhsT=qt_sb[:, ns,
                                           cm * 128:(cm + 1) * 128],
                                rhs=kt_sb[:, ns, :],
                                start=(ci == 0 and ns == 0 and cm == 0),
                                stop=(ci == N // CH_A - 1
                                      and ns == n_sub_a - 1),
                                skip_group_check=True)

                # ================= phase B: negated softmax ==============
                rmin = smax.tile([128, 2], f32, tag="rmin")
                rsum = smax.tile([128, 2], f32, tag="rsum")
                rinv = smax.tile([128, 2], f32, tag="rinv")
                pbvn = smax.tile([128, 2], f32, tag="pbvn")
                p_sb = smax.tile([128, 2, 256], f32, tag="p_sb")
                pscr = smax.tile([128, 2, 256], f32, tag="pscr")
                for cm in range(2):
                    nc.vector.tensor_reduce(
                        out=rmin[:, cm:cm + 1], in_=e_ps[:, cm, :],
                        axis=AX.X, op=OP.min)
                    # P = exp(-energy + rowmin), rowsum fused
                    nc.scalar.activation(
                        out=p_sb[:, cm, :], in_=e_ps[:, cm, :], func=AF.Exp,
                        bias=rmin[:, cm:cm + 1], scale=-1.0,
                        accum_out=rsum[:, cm:cm + 1])
                nc.vector.reciprocal(rinv[:, :], rsum[:, :])
                # att = P * rinv (normalized attention, per-partition scalar)
                att = smax.tile([128, 2, 256], f32, tag="att")
                for cm in range(2):
                    nc.vector.tensor_scalar_mul(
                        att[:, cm, :], p_sb[:, cm, :], rinv[:, cm:cm + 1])
                # pbvn = att @ bv (elementwise mul then row-reduce on DVE)
                for cm in range(2):
                    nc.vector.tensor_tensor(
                        out=pscr[:, cm, :], in0=att[:, cm, :],
                        in1=bvb[:, :], op=OP.mult)
                    nc.vector.tensor_reduce(
                        out=pbvn[:, cm:cm + 1], in_=pscr[:, cm, :],
                        axis=AX.X, op=OP.add)
                # PT[d, c] via PE transpose of the four 128x128 blocks
                pt_ps = ps_e.tile([128, 2, 256], f32, tag="e")
                pt_sb = smax.tile([128, 2, 256], f32r, tag="pt_sb")
                for dt in range(2):
                    for cm in range(2):
                        nc.tensor.transpose(
                            out=pt_ps[:, dt, cm * 128:(cm + 1) * 128],
                            in_=att[:, cm, dt * 128:(dt + 1) * 128],
                            identity=ident[:, :])
                nc.vector.tensor_copy(pt_sb[:, :, :], pt_ps[:, :, :])
                # G^T[j, c] = sum_f Wv[f, j] * PT[f, c]  (G = P @ Wv)
                gt_ps = ps_g.tile([128, 2, 256], f32, tag="g")
                gt_sb = smax.tile([128, 2, 256], f32r, tag="gt_sb")
                for jt in range(2):
                    for ft in range(2):
                        nc.tensor.matmul(
                            gt_ps[:, jt, :],
                            lhsT=wv[:, ft, jt * 128:(jt + 1) * 128],
                            rhs=pt_sb[:, ft, :],
                            start=(ft == 0), stop=(ft == 1))
                nc.vector.tensor_copy(gt_sb[:, :, :], gt_ps[:, :, :])

                # == phase D: stream x; out = (G@x + pbv)*rinv ============
                for cd in range(N // CH_DX):
                    off = cd * CH_DX
                    xc = xc_p.tile([128, 2, CH_DX], f32r, tag="xc")
                    x_eng = nc.sync if cd % 2 == 0 else nc.scalar
                    x_eng.dma_start(
                        out=xc[:, :, :],
                        in_=x_in[b, :, off:off + CH_DX].rearrange(
                            "(t p) n -> p t n", p=128))
                    o_sb = osb.tile([128, 2, CH_DO], f32)
                    for sub in range(CH_DX // CH_A):
                        so = sub * CH_A
                        for cm in range(2):
                            o_ps = ps_o.tile([128, CH_A], f32)
                            for jt in range(2):
                                nc.tensor.matmul(
                                    o_ps[:, :],
                                    lhsT=gt_sb[:, jt,
                                               cm * 128:(cm + 1) * 128],
                                    rhs=xc[:, jt, so:so + CH_A],
                                    start=(jt == 0), stop=(jt == 1))
                            # out = o + pbvn  (normalization folded into G)
                            nc.vector.tensor_scalar_add(
                                out=o_sb[:, cm, so:so + CH_A],
                                in0=o_ps[:, :],
                                scalar1=pbvn[:, cm:cm + 1])
                    nc.gpsimd.dma_start(
                        out=out_d[b, :, off:off + CH_DO].rearrange(
                            "(t p) n -> p t n", p=128),
                        in_=o_sb[:, :, :])
    if not nc.is_finalized():
        nc.finalize()
    return nc


def make_in_maps(query, key, x, Wq, bq, Wk, bk, Wv, bv):
    query = np.ascontiguousarray(np.asarray(query, dtype=np.float32))
    key = np.ascontiguousarray(np.asarray(key, dtype=np.float32))
    x = np.ascontiguousarray(np.asarray(x, dtype=np.float32))
    Wq = np.asarray(Wq, dtype=np.float32)
    bq = np.asarray(bq, dtype=np.float32)
    Wk = np.asarray(Wk, dtype=np.float32)
    bk = np.asarray(bk, dtype=np.float32)
    Wv = np.asarray(Wv, dtype=np.float32)
    bv = np.asarray(bv, dtype=np.float32)

    B, Cc, H, W = query.shape
    assert (B, Cc, H * W) == (B_FULL, C, N)

    consts = {
        "wqt": np.ascontiguousarray(Wq.T),
        "wkt": np.ascontiguousarray(Wk.T),
        "wv": np.ascontiguousarray(Wv),
        "bqb": np.ascontiguousarray(
            np.broadcast_to(bq[None, None, :], (128, 4, 256))),
        "bkb": np.ascontiguousarray(
            np.broadcast_to(bk[None, None, :], (128, 4, 256))),
        "bvb": np.ascontiguousarray(
            np.broadcast_to(bv[None, :], (128, 256))),
        "ident": np.eye(128, dtype=np.float32),
    }
    in_maps = []
    for i in range(N_CORES):
        sl = slice(i * B2, (i + 1) * B2)
        in_maps.append({
            "q_in": query[sl].reshape(B2, C, N),
            "k_in": key[sl].reshape(B2, C, N),
            "x_in": x[sl].reshape(B2, C, N),
            **consts,
        })
    return in_maps


def kernel(query, key, x, Wq, bq, Wk, bk, Wv, bv):
    from concourse.bass_utils import run_bass_kernel_spmd

    in_maps = make_in_maps(query, key, x, Wq, bq, Wk, bk, Wv, bv)

    if "nc" not in _CACHE:
        _CACHE["nc"] = _build()
    nc = _CACHE["nc"]

    res = run_bass_kernel_spmd(nc, in_maps, list(range(N_CORES)))
    out = np.concatenate([res.results[i]["out"] for i in range(N_CORES)], axis=0)
    return out.reshape(B_FULL, C, N // 128, 128).astype(np.float32)
